# revision 5
# baseline (speedup 1.0000x reference)
"""Trainium2 Bass kernel for nn_MemoryReader (retrieval_knn).

Math (per batch b):
  mk_h [h,c,n] (c=16, n=THW=8192), qk_h/qe_h [h,c,m] (m=HW=1024)
  logits[h,n,m] = (ms[n]/8) * ( sum_c mk^3*(-qe) + mk*(2*qk*qe) + (-b_sq) )
  aff = softmax over h
  mem[h,c',m] = sum_n mo[h,c',n] * aff[h,n,m]   (c'=128)
  out = concat(mem, qv)

Sharding: 8 cores = 2 batches x 4 THW-chunks (n-chunk 2048/core). Softmax is
over heads -> core-local. Readout partial-sums over n are reduced on host
during the gather (legit unshard of a contraction-sharded axis).

Device kernel per core:
  x  [33, 4*2048]  : per head [mk^3*msn; mk*msn; msn] (msn = ms/8 folded in,
                     row 32 of ones*msn folds the -b_sq term via w row 32)
  w  [33, 4*1024]  : per head [-qe; 2*qk*qe; -b_sq]
  mvt[2048, 512]   : mv chunk transposed (n on partitions for readout matmul)
  -> sim matmul (K=33, bf16) -> exp (ACT) -> sum (DVE) / sum (GPS) / recip
     (DVE custom NR) / mul (DVE) -> readout matmul accumulating over the 16
     n-tiles in PSUM -> mem [512,1024]

Pipeline: readout matmuls are emitted LAG=2 iterations behind the softmax
chain that produces their aff operand, so the PE never stalls on the
ACT->DVE->GPS->DVE chain latency (it runs the next iterations' sims
instead); this also keeps PE activity dense enough for HAM to hold the
2.4GHz clock (K=8/8) instead of oscillating to 1.2GHz.
"""

import sys

sys.path.insert(0, "/opt/trn_rl_repo")

import numpy as np

import concourse.bass as bass
import concourse.tile as tile
from concourse import bacc, mybir
from concourse.bass_utils import run_bass_kernel_spmd

try:
    import ml_dtypes

    _BF16_NP = np.dtype(ml_dtypes.bfloat16)
except ImportError:  # pragma: no cover
    _BF16_NP = None

HEADS, B, CK, CV = 4, 2, 64, 512
T, H, W = 8, 32, 32
THW, HW = T * H * W, H * W          # 8192, 1024
C = CK // HEADS                      # 16
NCHUNK = THW // 4                    # 2048 n per core
NT = NCHUNK // 128                   # 16 n-tiles per core
KDIM = 2 * C + 1                     # 33
NITER = 2 * NT                       # (mh, nt) flat iteration count
LAG = 2                              # readout lag (iterations)

F32 = mybir.dt.float32
BF16 = mybir.dt.bfloat16

SIM_DT = BF16            # x/w dtype (bf16: 1 cyc/col at every PE p-state)
EW_DT = BF16             # dtype of e/aff (softmax elementwise) + mvt


def _np_dt(dt):
    return _BF16_NP if dt == BF16 else np.float32


def build_bass():
    # Bacc (not plain Bass): its compile()/finalize() pipeline legalizes
    # multi-wait instructions (TRN2 allows 1 wait/inst) via event semaphores.
    nc = bacc.Bacc(None)
    # xw row-tiled layout: partitions 0-63 hold heads {0,2} (33 real rows,
    # zero-padded to 64), partitions 64-127 hold heads {1,3}. Head pair
    # (2p, 2p+1) runs as two CONCURRENT K=64 matmuls via tile_position
    # (0,0)/(64,0) -- halves sim streaming time on the PE.
    PB = NCHUNK + HW  # per-pair free block: [X 2048 | W 1024]
    xw_d = nc.dram_tensor("xw", [128, 2 * PB], SIM_DT, kind="ExternalInput")
    mvt_d = nc.dram_tensor("mvt", [NCHUNK, CV], EW_DT, kind="ExternalInput")
    mem_d = nc.dram_tensor("mem", [CV, HW], F32, kind="ExternalOutput")

    Exp = mybir.ActivationFunctionType.Exp
    Copy = mybir.ActivationFunctionType.Copy

    from concourse.dve_ops import (
        RECIP_APPROX_FAST_CONSTS as _RC,
        RECIPROCAL_APPROX_FAST as _RF,
    )

    with tile.TileContext(nc) as tc:
        with (
            tc.tile_pool(name="const", bufs=1) as constp,
            tc.tile_pool(name="simp", bufs=2, space="PSUM") as simp,
            tc.tile_pool(name="memp", bufs=1, space="PSUM") as memp,
            tc.tile_pool(name="work", bufs=6) as work,
            tc.tile_pool(name="outp", bufs=2) as outp,
        ):
            xw_sb = constp.tile([128, 2 * PB], SIM_DT)
            # Interleave pair-0/pair-1 chunks (W halves first, then X
            # quarters) so BOTH pairs' first tiles arrive early — the first
            # iteration needs pr0 and pr1 data.
            for wh in range(2):
                for pr in range(2):
                    o = pr * PB + NCHUNK + wh * 512
                    nc.sync.dma_start(
                        out=xw_sb[:, o : o + 512], in_=xw_d[:, o : o + 512]
                    )
            for xh in range(4):
                for pr in range(2):
                    o = pr * PB + xh * (NCHUNK // 4)
                    nc.sync.dma_start(
                        out=xw_sb[:, o : o + NCHUNK // 4],
                        in_=xw_d[:, o : o + NCHUNK // 4],
                    )
            mvt_sb = constp.tile([128, NT * CV], EW_DT)
            for nt in range(NT):
                nc.sync.dma_start(
                    out=mvt_sb[:, nt * CV : (nt + 1) * CV],
                    in_=mvt_d[nt * 128 : (nt + 1) * 128, :],
                )

            # Heater: back-to-back dummy MMs warm the PE (HAM) before the
            # loop. Source is a memset tile (not DMA'd data) so the heater
            # runs DURING the input-DMA wait instead of after it, and the PE
            # is already at K=8/8 when the first sims arrive.
            hsrc = constp.tile([64, 768], BF16)
            nc.vector.memset(hsrc[:], 0.0)
            warm = simp.tile([128, 1024], F32, tag="sim")
            for _ in range(10):
                wmm = nc.tensor.matmul(
                    warm[:, :512],
                    lhsT=hsrc[:, 0:128],
                    rhs=hsrc[:, 128:640],
                    start=True,
                    stop=True,
                    tile_position=(0, 0),
                )
                wmm.ins.bass_priority = -100  # pin to the front of the PE queue

            aff_tiles = {}
            mem_tiles = {}
            for it in range(NITER + LAG):
                if it < NITER:
                    mh, nt = divmod(it, NT)
                    # --- similarity logits: 4 heads, K=33, N=512 ---
                    simA = simp.tile([128, 1024], F32, tag="sim")
                    simB = simp.tile([128, 1024], F32, tag="sim")
                    # Drip heater: a tiny MM into the slice simA is about to
                    # overwrite (start=True clears it) keeps the PE's HAM
                    # activity window busy enough to hold the K=8/8 clock
                    # (2.4GHz) through the softmax-chain gaps.
                    nc.tensor.matmul(
                        simA[:, 0:64],
                        lhsT=hsrc[:, 0:128],
                        rhs=hsrc[:, 128:192],
                        start=True,
                        stop=True,
                        tile_position=(0, 0),
                    )
                    for pr in range(2):
                        ps = simA if pr == 0 else simB
                        for half in range(2):
                            base = half * 64
                            nc.tensor.matmul(
                                ps[:, half * 512 : half * 512 + 512],
                                lhsT=xw_sb[base : base + 64,
                                           pr * PB + nt * 128 : pr * PB + nt * 128 + 128],
                                rhs=xw_sb[base : base + 64,
                                          pr * PB + NCHUNK + mh * 512 : pr * PB + NCHUNK + mh * 512 + 512],
                                start=True,
                                stop=True,
                                tile_position=(base, 0),
                            )
                    # --- softmax over heads (no max-sub: |logit| <= ~25) ---
                    e_all = work.tile([128, 2048], EW_DT, tag="e")
                    nc.scalar.activation(e_all[:, :1024], simA[:], Exp)
                    nc.scalar.activation(e_all[:, 1024:], simB[:], Exp)
                    # sp = e01 + e23, split across DVE (lo half) and GPSIMD
                    # (hi half) to take ~0.35us/iter off the DVE pacer.
                    sp = work.tile([128, 1024], EW_DT, tag="sp")
                    nc.vector.tensor_add(
                        sp[:, :512], e_all[:, :512], e_all[:, 1024:1536]
                    )
                    nc.gpsimd.tensor_add(
                        sp[:, 512:], e_all[:, 512:1024], e_all[:, 1536:]
                    )
                    s_f = work.tile([128, 512], F32, tag="S")
                    nc.gpsimd.tensor_add(s_f[:], sp[:, :512], sp[:, 512:])
                    # custom NR reciprocal writing bf16 directly (out-dtype
                    # conversion happens at the DVE write port) — saves the
                    # separate f32->bf16 cast op.
                    r_use = work.tile([128, 512], EW_DT, tag="Rb")
                    nc.vector._custom_dve(
                        _RF,
                        out=r_use[:],
                        in0=s_f[:],
                        s0=_RC["s0"],
                        s1=_RC["s1"],
                        imm2=_RC["imm2"],
                    )
                    aff = work.tile([128, 4 * 512], EW_DT, tag="aff")
                    nc.vector.tensor_mul(
                        aff.rearrange("p (h m) -> p h m", h=4),
                        e_all.rearrange("p (h m) -> p h m", h=4),
                        r_use[:, None, :].to_broadcast((128, 4, 512)),
                    )
                    aff_tiles[it] = aff
                # --- readout: LAG iterations behind, accumulate over nt ---
                ro = it - LAG
                if 0 <= ro < NITER:
                    mh_r, nt_r = divmod(ro, NT)
                    if nt_r == 0:
                        mem_tiles[mh_r] = memp.tile(
                            [128, 4 * 512], F32, tag="mem", name=f"mem_ps{mh_r}"
                        )
                    mem_ps = mem_tiles[mh_r]
                    aff_r = aff_tiles.pop(ro)
                    for h in range(HEADS):
                        nc.tensor.matmul(
                            mem_ps[:, h * 512 : (h + 1) * 512],
                            lhsT=mvt_sb[:, nt_r * CV + h * 128 : nt_r * CV + h * 128 + 128],
                            rhs=aff_r[:, h * 512 : (h + 1) * 512],
                            start=(nt_r == 0),
                            stop=(nt_r == NT - 1),
                        )
                    if nt_r == NT - 1:
                        mem_sb = outp.tile([128, 4 * 512], F32)
                        for h in range(HEADS):
                            # per-head copy, alternating ACT/DVE, so each
                            # output DMA starts as soon as its slice is
                            # staged (shorter kernel tail)
                            dst = mem_sb[:, h * 512 : (h + 1) * 512]
                            src = mem_ps[:, h * 512 : (h + 1) * 512]
                            if h % 2 == 0:
                                nc.scalar.activation(dst, src, Copy)
                            else:
                                nc.vector.tensor_copy(dst, src)
                            nc.sync.dma_start(
                                out=mem_d[h * 128 : (h + 1) * 128,
                                          mh_r * 512 : (mh_r + 1) * 512],
                                in_=mem_sb[:, h * 512 : (h + 1) * 512],
                            )
    return nc


def host_decompose(mk, qk, ms, qe, mv):
    """Build the 8 per-core input dicts."""
    mk_f = np.asarray(mk, np.float32).reshape(B, CK, THW)
    mv_f = np.asarray(mv, np.float32).reshape(B, CV, THW)
    ms_f = np.asarray(ms, np.float32).reshape(B, THW)
    qk_h = np.asarray(qk, np.float32).reshape(B, HEADS, C, HW)
    qe_h = np.asarray(qe, np.float32).reshape(B, HEADS, C, HW)

    msn = ms_f / np.float32(np.sqrt(CK))                       # [B, THW]
    mk3 = mk_f * mk_f * mk_f                                   # [B, CK, THW]

    # w [B, 33, h, m]
    w_all = np.empty((B, KDIM, HEADS, HW), np.float32)
    w_all[:, :C] = -np.swapaxes(qe_h, 1, 2)
    w_all[:, C : 2 * C] = np.swapaxes(2.0 * qk_h * qe_h, 1, 2)
    w_all[:, 2 * C] = -np.sum(qe_h * qk_h**3, axis=2)

    # x [B, 33, h, n]
    x_all = np.empty((B, KDIM, HEADS, THW), np.float32)
    mk3_h = mk3.reshape(B, HEADS, C, THW)
    mk_h = mk_f.reshape(B, HEADS, C, THW)
    x_all[:, :C] = np.swapaxes(mk3_h, 1, 2) * msn[:, None, None, :]
    x_all[:, C : 2 * C] = np.swapaxes(mk_h, 1, 2) * msn[:, None, None, :]
    x_all[:, 2 * C] = msn[:, None, :]

    sim_np = _np_dt(SIM_DT)
    mvt_np = _np_dt(EW_DT)
    PB = NCHUNK + HW
    in_maps = []
    for core in range(8):
        b, j = core // 4, core % 4
        sl = slice(j * NCHUNK, (j + 1) * NCHUNK)
        xw = np.zeros((128, 2 * PB), sim_np)
        for pr in range(2):
            for half in range(2):
                h = 2 * pr + half
                r0 = half * 64
                xw[r0 : r0 + KDIM, pr * PB : pr * PB + NCHUNK] = x_all[b, :, h, sl].astype(sim_np)
                xw[r0 : r0 + KDIM, pr * PB + NCHUNK : (pr + 1) * PB] = w_all[b, :, h].astype(sim_np)
        mvt = np.ascontiguousarray(mv_f[b, :, sl].T).astype(mvt_np)
        in_maps.append({"xw": xw, "mvt": mvt})
    return in_maps


_NC_CACHE = None


def _get_nc():
    global _NC_CACHE
    if _NC_CACHE is None:
        nc = build_bass()
        if not nc.is_finalized():
            nc.finalize()  # Bacc compile: wait legalization etc.
        _NC_CACHE = nc
    return _NC_CACHE


def kernel(mk, qk, ms, qe, mv, qv, _trace=False, _trace_kwargs=None):
    in_maps = host_decompose(mk, qk, ms, qe, mv)
    nc = _get_nc()
    res = run_bass_kernel_spmd(
        nc, in_maps, list(range(8)), trace=_trace, **(_trace_kwargs or {})
    )
    mem = np.zeros((B, CV, HW), np.float32)
    for core in range(8):
        mem[core // 4] += res.results[core]["mem"]
    out = np.concatenate(
        [mem.reshape(B, CV, H, W), np.asarray(qv, np.float32).reshape(B, CV, H, W)],
        axis=1,
    )
    if _trace:
        return out, res
    return out


# revision 13
# speedup vs baseline: 1.1094x; 1.1094x over previous
"""Trainium2 Bass kernel for nn_MemoryReader (retrieval_knn).

Math (per batch b):
  mk_h [h,c,n] (c=16, n=THW=8192), qk_h/qe_h [h,c,m] (m=HW=1024)
  logits[h,n,m] = (ms[n]/8) * ( sum_c mk^3*(-qe) + mk*(2*qk*qe) + (-b_sq) )
  aff = softmax over h
  mem[h,c',m] = sum_n mo[h,c',n] * aff[h,n,m]   (c'=128)
  out = concat(mem, qv)

Sharding: 8 cores = 2 batches x 4 THW-chunks (n-chunk 2048/core). Softmax is
over heads -> core-local. Readout partial-sums over n are reduced on host
during the gather (legit unshard of a contraction-sharded axis).

Device kernel per core:
  x  [33, 4*2048]  : per head [mk^3*msn; mk*msn; msn] (msn = ms/8 folded in,
                     row 32 of ones*msn folds the -b_sq term via w row 32)
  w  [33, 4*1024]  : per head [-qe; 2*qk*qe; -b_sq]
  mvt[2048, 512]   : mv chunk transposed (n on partitions for readout matmul)
  -> sim matmul (K=33, bf16) -> exp (ACT) -> sum (DVE) / sum (GPS) / recip
     (DVE custom NR) / mul (DVE) -> readout matmul accumulating over the 16
     n-tiles in PSUM -> mem [512,1024]

Pipeline: readout matmuls are emitted LAG=2 iterations behind the softmax
chain that produces their aff operand, so the PE never stalls on the
ACT->DVE->GPS->DVE chain latency (it runs the next iterations' sims
instead); this also keeps PE activity dense enough for HAM to hold the
2.4GHz clock (K=8/8) instead of oscillating to 1.2GHz.
"""

import sys

sys.path.insert(0, "/opt/trn_rl_repo")

import numpy as np

import concourse.bass as bass
import concourse.tile as tile
from concourse import bacc, mybir
from concourse.bass_utils import run_bass_kernel_spmd

try:
    import ml_dtypes

    _BF16_NP = np.dtype(ml_dtypes.bfloat16)
except ImportError:  # pragma: no cover
    _BF16_NP = None

HEADS, B, CK, CV = 4, 2, 64, 512
T, H, W = 8, 32, 32
THW, HW = T * H * W, H * W          # 8192, 1024
C = CK // HEADS                      # 16
NCHUNK = THW // 4                    # 2048 n per core
NT = NCHUNK // 128                   # 16 n-tiles per core
KDIM = 2 * C + 1                     # 33
NITER = 2 * NT                       # (mh, nt) flat iteration count
LAG = 3                              # readout lag (iterations)

F32 = mybir.dt.float32
BF16 = mybir.dt.bfloat16

SIM_DT = BF16            # x/w dtype (bf16: 1 cyc/col at every PE p-state)
EW_DT = BF16             # dtype of e/aff (softmax elementwise) + mvt


def _np_dt(dt):
    return _BF16_NP if dt == BF16 else np.float32


def build_bass():
    # Bacc (not plain Bass): its compile()/finalize() pipeline legalizes
    # multi-wait instructions (TRN2 allows 1 wait/inst) via event semaphores.
    nc = bacc.Bacc(None)
    # xw row-tiled layout: partitions 0-63 hold heads {0,2} (33 real rows,
    # zero-padded to 64), partitions 64-127 hold heads {1,3}. Head pair
    # (2p, 2p+1) runs as two CONCURRENT K=64 matmuls via tile_position
    # (0,0)/(64,0) -- halves sim streaming time on the PE.
    PB = NCHUNK + HW  # per-pair free block: [X 2048 | W 1024]
    xw_d = nc.dram_tensor("xw", [128, 2 * PB], SIM_DT, kind="ExternalInput")
    mvt_d = nc.dram_tensor("mvt", [NCHUNK, CV], EW_DT, kind="ExternalInput")
    mem_d = nc.dram_tensor("mem", [CV, HW], F32, kind="ExternalOutput")

    Exp = mybir.ActivationFunctionType.Exp
    Copy = mybir.ActivationFunctionType.Copy

    from concourse.dve_ops import (
        RECIP_APPROX_FAST_CONSTS as _RC,
        RECIPROCAL_APPROX_FAST as _RF,
    )

    with tile.TileContext(nc) as tc:
        with (
            tc.tile_pool(name="const", bufs=1) as constp,
            tc.tile_pool(name="simp", bufs=2, space="PSUM") as simp,
            tc.tile_pool(name="memp", bufs=1, space="PSUM") as memp,
            tc.tile_pool(name="work", bufs=6) as work,
            tc.tile_pool(name="outp", bufs=2) as outp,
        ):
            xw_sb = constp.tile([128, 2 * PB], SIM_DT)
            # Interleave pair-0/pair-1 chunks (W halves first, then X
            # quarters) so BOTH pairs' first tiles arrive early — the first
            # iteration needs pr0 and pr1 data.
            for wh in range(2):
                for pr in range(2):
                    o = pr * PB + NCHUNK + wh * 512
                    nc.sync.dma_start(
                        out=xw_sb[:, o : o + 512], in_=xw_d[:, o : o + 512]
                    )
            for xh in range(4):
                for pr in range(2):
                    o = pr * PB + xh * (NCHUNK // 4)
                    nc.sync.dma_start(
                        out=xw_sb[:, o : o + NCHUNK // 4],
                        in_=xw_d[:, o : o + NCHUNK // 4],
                    )
            mvt_sb = constp.tile([128, NT * CV], EW_DT)
            for nt in range(NT):
                nc.sync.dma_start(
                    out=mvt_sb[:, nt * CV : (nt + 1) * CV],
                    in_=mvt_d[nt * 128 : (nt + 1) * 128, :],
                )

            # Heater: back-to-back dummy MMs warm the PE (HAM) before the
            # loop. Source is a memset tile (not DMA'd data) so the heater
            # runs DURING the input-DMA wait instead of after it, and the PE
            # is already at K=8/8 when the first sims arrive.
            hsrc = constp.tile([64, 768], BF16)
            nc.vector.memset(hsrc[:], 0.0)
            warm = simp.tile([128, 1024], F32, tag="sim")
            for _ in range(10):
                wmm = nc.tensor.matmul(
                    warm[:, :512],
                    lhsT=hsrc[:, 0:128],
                    rhs=hsrc[:, 128:640],
                    start=True,
                    stop=True,
                    tile_position=(0, 0),
                )
                wmm.ins.bass_priority = -100  # pin to the front of the PE queue

            aff_tiles = {}
            mem_tiles = {}
            for it in range(NITER + LAG):
                if it < NITER:
                    mh, nt = divmod(it, NT)
                    # --- similarity logits: 4 heads, K=33, N=512 ---
                    simA = simp.tile([128, 1024], F32, tag="sim")
                    simB = simp.tile([128, 1024], F32, tag="sim")
                    # Drip heater: a small MM into the slice simA is about to
                    # overwrite (start=True clears it anyway) keeps the PE's
                    # HAM activity window busy enough to hold the K=8/8 clock
                    # (2.4GHz). LDWEIGHTS-only drips do NOT register as HAM
                    # activity (measured) — it has to be a real matmul.
                    nc.tensor.matmul(
                        simA[:, 0:256],
                        lhsT=hsrc[:, 0:128],
                        rhs=hsrc[:, 128:384],
                        start=True,
                        stop=True,
                        tile_position=(0, 0),
                    )
                    for pr in range(2):
                        ps = simA if pr == 0 else simB
                        for half in range(2):
                            base = half * 64
                            nc.tensor.matmul(
                                ps[:, half * 512 : half * 512 + 512],
                                lhsT=xw_sb[base : base + 64,
                                           pr * PB + nt * 128 : pr * PB + nt * 128 + 128],
                                rhs=xw_sb[base : base + 64,
                                          pr * PB + NCHUNK + mh * 512 : pr * PB + NCHUNK + mh * 512 + 512],
                                start=True,
                                stop=True,
                                tile_position=(base, 0),
                            )
                    # --- softmax over heads (no max-sub: |logit| <= ~25) ---
                    e_all = work.tile([128, 2048], EW_DT, tag="e")
                    nc.scalar.activation(e_all[:, :1024], simA[:], Exp)
                    nc.scalar.activation(e_all[:, 1024:], simB[:], Exp)
                    sp = work.tile([128, 1024], EW_DT, tag="sp")
                    nc.vector.tensor_add(sp[:], e_all[:, :1024], e_all[:, 1024:])
                    s_f = work.tile([128, 512], F32, tag="S")
                    nc.gpsimd.tensor_add(s_f[:], sp[:, :512], sp[:, 512:])
                    # custom NR reciprocal writing bf16 directly (out-dtype
                    # conversion happens at the DVE write port) — saves the
                    # separate f32->bf16 cast op.
                    r_use = work.tile([128, 512], EW_DT, tag="Rb")
                    nc.vector._custom_dve(
                        _RF,
                        out=r_use[:],
                        in0=s_f[:],
                        s0=_RC["s0"],
                        s1=_RC["s1"],
                        imm2=_RC["imm2"],
                    )
                    aff = work.tile([128, 4 * 512], EW_DT, tag="aff")
                    nc.vector.tensor_mul(
                        aff.rearrange("p (h m) -> p h m", h=4),
                        e_all.rearrange("p (h m) -> p h m", h=4),
                        r_use[:, None, :].to_broadcast((128, 4, 512)),
                    )
                    aff_tiles[it] = aff
                # --- readout: LAG iterations behind, accumulate over nt ---
                ro = it - LAG
                if 0 <= ro < NITER:
                    mh_r, nt_r = divmod(ro, NT)
                    if nt_r == 0:
                        mem_tiles[mh_r] = memp.tile(
                            [128, 4 * 512], F32, tag="mem", name=f"mem_ps{mh_r}"
                        )
                    mem_ps = mem_tiles[mh_r]
                    aff_r = aff_tiles.pop(ro)
                    for h in range(HEADS):
                        nc.tensor.matmul(
                            mem_ps[:, h * 512 : (h + 1) * 512],
                            lhsT=mvt_sb[:, nt_r * CV + h * 128 : nt_r * CV + h * 128 + 128],
                            rhs=aff_r[:, h * 512 : (h + 1) * 512],
                            start=(nt_r == 0),
                            stop=(nt_r == NT - 1),
                        )

                    if nt_r == NT - 1:
                        mem_sb = outp.tile([128, 4 * 512], F32)
                        for h in range(HEADS):
                            # per-head copy, alternating ACT/DVE, so each
                            # output DMA starts as soon as its slice is
                            # staged (shorter kernel tail)
                            dst = mem_sb[:, h * 512 : (h + 1) * 512]
                            src = mem_ps[:, h * 512 : (h + 1) * 512]
                            if h % 2 == 0:
                                nc.scalar.activation(dst, src, Copy)
                            else:
                                nc.vector.tensor_copy(dst, src)
                            nc.sync.dma_start(
                                out=mem_d[h * 128 : (h + 1) * 128,
                                          mh_r * 512 : (mh_r + 1) * 512],
                                in_=mem_sb[:, h * 512 : (h + 1) * 512],
                            )
    return nc


def host_decompose(mk, qk, ms, qe, mv):
    """Build the 8 per-core input dicts."""
    mk_f = np.asarray(mk, np.float32).reshape(B, CK, THW)
    mv_f = np.asarray(mv, np.float32).reshape(B, CV, THW)
    ms_f = np.asarray(ms, np.float32).reshape(B, THW)
    qk_h = np.asarray(qk, np.float32).reshape(B, HEADS, C, HW)
    qe_h = np.asarray(qe, np.float32).reshape(B, HEADS, C, HW)

    msn = ms_f / np.float32(np.sqrt(CK))                       # [B, THW]
    mk3 = mk_f * mk_f * mk_f                                   # [B, CK, THW]

    # w [B, 33, h, m]
    w_all = np.empty((B, KDIM, HEADS, HW), np.float32)
    w_all[:, :C] = -np.swapaxes(qe_h, 1, 2)
    w_all[:, C : 2 * C] = np.swapaxes(2.0 * qk_h * qe_h, 1, 2)
    w_all[:, 2 * C] = -np.sum(qe_h * qk_h**3, axis=2)

    # x [B, 33, h, n]
    x_all = np.empty((B, KDIM, HEADS, THW), np.float32)
    mk3_h = mk3.reshape(B, HEADS, C, THW)
    mk_h = mk_f.reshape(B, HEADS, C, THW)
    x_all[:, :C] = np.swapaxes(mk3_h, 1, 2) * msn[:, None, None, :]
    x_all[:, C : 2 * C] = np.swapaxes(mk_h, 1, 2) * msn[:, None, None, :]
    x_all[:, 2 * C] = msn[:, None, :]

    sim_np = _np_dt(SIM_DT)
    mvt_np = _np_dt(EW_DT)
    PB = NCHUNK + HW
    in_maps = []
    for core in range(8):
        b, j = core // 4, core % 4
        sl = slice(j * NCHUNK, (j + 1) * NCHUNK)
        xw = np.zeros((128, 2 * PB), sim_np)
        for pr in range(2):
            for half in range(2):
                h = 2 * pr + half
                r0 = half * 64
                xw[r0 : r0 + KDIM, pr * PB : pr * PB + NCHUNK] = x_all[b, :, h, sl].astype(sim_np)
                xw[r0 : r0 + KDIM, pr * PB + NCHUNK : (pr + 1) * PB] = w_all[b, :, h].astype(sim_np)
        mvt = np.ascontiguousarray(mv_f[b, :, sl].T).astype(mvt_np)
        in_maps.append({"xw": xw, "mvt": mvt})
    return in_maps


_NC_CACHE = None


def _get_nc():
    global _NC_CACHE
    if _NC_CACHE is None:
        nc = build_bass()
        if not nc.is_finalized():
            nc.finalize()  # Bacc compile: wait legalization etc.
        _NC_CACHE = nc
    return _NC_CACHE


def kernel(mk, qk, ms, qe, mv, qv, _trace=False, _trace_kwargs=None):
    in_maps = host_decompose(mk, qk, ms, qe, mv)
    nc = _get_nc()
    res = run_bass_kernel_spmd(
        nc, in_maps, list(range(8)), trace=_trace, **(_trace_kwargs or {})
    )
    mem = np.zeros((B, CV, HW), np.float32)
    for core in range(8):
        mem[core // 4] += res.results[core]["mem"]
    out = np.concatenate(
        [mem.reshape(B, CV, H, W), np.asarray(qv, np.float32).reshape(B, CV, H, W)],
        axis=1,
    )
    if _trace:
        return out, res
    return out


# revision 15
# speedup vs baseline: 1.1178x; 1.0076x over previous
"""Trainium2 Bass kernel for nn_MemoryReader (retrieval_knn).

Math (per batch b):
  mk_h [h,c,n] (c=16, n=THW=8192), qk_h/qe_h [h,c,m] (m=HW=1024)
  logits[h,n,m] = (ms[n]/8) * ( sum_c mk^3*(-qe) + mk*(2*qk*qe) + (-b_sq) )
  aff = softmax over h
  mem[h,c',m] = sum_n mo[h,c',n] * aff[h,n,m]   (c'=128)
  out = concat(mem, qv)

Sharding: 8 cores = 2 batches x 4 THW-chunks (n-chunk 2048/core). Softmax is
over heads -> core-local. Readout partial-sums over n are reduced on host
during the gather (legit unshard of a contraction-sharded axis).

3-exp diff formulation: softmax is shift-invariant, so subtract head 0's
logits on the PE: d_h = sim_h - sim_0 (h=1..3) via K=65 matmuls
  x rows: [mk^3_h*msn(16); mk_h*msn(16); mk^3_0*msn(16); mk_0*msn(16); msn]
  w rows: [-qe_h; 2qk_h qe_h; +qe_0; -2qk_0 qe_0; b_0-b_h]
then aff_0 = 1/(1 + e^d1 + e^d2 + e^d3) comes straight out of the DVE
reciprocal (no mul for head 0), and only 3 exps run on ACT (the pacing
engine) instead of 4.

PSUM (8 banks): mem accumulate 4 + D12 2 + D3 1 + warm 1. The warm bank is
a dependency-free drip-heater target: one small MM per iteration keeps the
PE's HAM activity monitor at K=8/8 (2.4GHz) even across the exp-wait gaps
(it has no producers/consumers, so it runs exactly when the PE idles).

Pipeline: readout matmuls are emitted LAG=3 iterations behind the softmax
chain that produces their aff operand, so the PE never stalls on the
ACT->DVE->GPS->DVE chain latency.
"""

import sys

sys.path.insert(0, "/opt/trn_rl_repo")

import numpy as np

import concourse.bass as bass
import concourse.tile as tile
from concourse import bacc, mybir
from concourse.bass_utils import run_bass_kernel_spmd

try:
    import ml_dtypes

    _BF16_NP = np.dtype(ml_dtypes.bfloat16)
except ImportError:  # pragma: no cover
    _BF16_NP = None

HEADS, B, CK, CV = 4, 2, 64, 512
T, H, W = 8, 32, 32
THW, HW = T * H * W, H * W          # 8192, 1024
C = CK // HEADS                      # 16
NCHUNK = THW // 4                    # 2048 n per core
NT = NCHUNK // 128                   # 16 n-tiles per core
KD = 4 * C + 1                       # 65 rows per diff matmul
NITER = 2 * NT                       # (mh, nt) flat iteration count
LAG = 3                              # readout lag (iterations)
PB = NCHUNK + HW                     # per-diff block: [X 2048 | W 1024]

F32 = mybir.dt.float32
BF16 = mybir.dt.bfloat16

SIM_DT = BF16            # x/w dtype (bf16: 1 cyc/col at every PE p-state)
EW_DT = BF16             # dtype of e/aff (softmax elementwise) + mvt


def _np_dt(dt):
    return _BF16_NP if dt == BF16 else np.float32


def build_bass():
    # Bacc (not plain Bass): its compile()/finalize() pipeline legalizes
    # multi-wait instructions (TRN2 allows 1 wait/inst) via event semaphores.
    nc = bacc.Bacc(None)
    xw_d = nc.dram_tensor("xw", [KD, 3 * PB], SIM_DT, kind="ExternalInput")
    mvt_d = nc.dram_tensor("mvt", [NCHUNK, CV], EW_DT, kind="ExternalInput")
    mem_d = nc.dram_tensor("mem", [CV, HW], F32, kind="ExternalOutput")

    Exp = mybir.ActivationFunctionType.Exp
    Copy = mybir.ActivationFunctionType.Copy
    Add = mybir.AluOpType.add

    from concourse.dve_ops import (
        RECIP_APPROX_FAST_CONSTS as _RC,
        RECIPROCAL_APPROX_FAST as _RF,
    )

    with tile.TileContext(nc) as tc:
        with (
            tc.tile_pool(name="const", bufs=1) as constp,
            tc.tile_pool(name="simp", bufs=1, space="PSUM") as simp,
            tc.tile_pool(name="memp", bufs=1, space="PSUM") as memp,
            tc.tile_pool(name="work", bufs=6) as work,
            tc.tile_pool(name="outp", bufs=2) as outp,
        ):
            xw_sb = constp.tile([KD, 3 * PB], SIM_DT)
            # W blocks first so the first iteration's streams arrive early,
            # then X in nt-order quarters interleaved across the 3 diffs.
            for dh in range(3):
                o = dh * PB + NCHUNK
                nc.sync.dma_start(out=xw_sb[:, o : o + HW], in_=xw_d[:, o : o + HW])
            for xq in range(4):
                for dh in range(3):
                    o = dh * PB + xq * (NCHUNK // 4)
                    nc.sync.dma_start(
                        out=xw_sb[:, o : o + NCHUNK // 4],
                        in_=xw_d[:, o : o + NCHUNK // 4],
                    )
            mvt_sb = constp.tile([128, NT * CV], EW_DT)
            for nt in range(NT):
                nc.sync.dma_start(
                    out=mvt_sb[:, nt * CV : (nt + 1) * CV],
                    in_=mvt_d[nt * 128 : (nt + 1) * 128, :],
                )

            # Startup heater + dedicated drip bank (no producers/consumers:
            # drip MMs only WAW-chain among themselves on the PE queue).
            hsrc = constp.tile([64, 768], BF16)
            nc.vector.memset(hsrc[:], 0.0)
            warm = simp.tile([128, 512], F32, tag="warm")
            for _ in range(10):
                wmm = nc.tensor.matmul(
                    warm[:],
                    lhsT=hsrc[:, 0:128],
                    rhs=hsrc[:, 128:640],
                    start=True,
                    stop=True,
                    tile_position=(0, 0),
                )
                wmm.ins.bass_priority = -100  # pin to the front of the PE queue

            aff_tiles = {}
            mem_tiles = {}
            for it in range(NITER + LAG):
                if it < NITER:
                    mh, nt = divmod(it, NT)
                    # dependency-free drip MM (see module docstring)
                    nc.tensor.matmul(
                        warm[:, 0:128],
                        lhsT=hsrc[:, 0:128],
                        rhs=hsrc[:, 128:256],
                        start=True,
                        stop=True,
                        tile_position=(0, 0),
                    )
                    # --- diff logits d_h = sim_h - sim_0: 3 MMs, K=65 ---
                    d12 = simp.tile([128, 1024], F32, tag="d12")
                    d3 = simp.tile([128, 512], F32, tag="d3")
                    for dh in range(3):
                        out_sl = d12[:, dh * 512 : dh * 512 + 512] if dh < 2 else d3[:]
                        nc.tensor.matmul(
                            out_sl,
                            lhsT=xw_sb[:, dh * PB + nt * 128 : dh * PB + nt * 128 + 128],
                            rhs=xw_sb[:, dh * PB + NCHUNK + mh * 512 : dh * PB + NCHUNK + mh * 512 + 512],
                            start=True,
                            stop=True,
                        )
                    # --- softmax over heads (aff_0 = 1/(1+sum e^d)) ---
                    e_all = work.tile([128, 1536], EW_DT, tag="e")
                    nc.scalar.activation(e_all[:, :1024], d12[:], Exp)
                    nc.scalar.activation(e_all[:, 1024:], d3[:], Exp)
                    sp = work.tile([128, 512], EW_DT, tag="sp")
                    nc.vector.tensor_add(sp[:], e_all[:, :512], e_all[:, 512:1024])
                    # s = 1 + sp + e3 (GPS add, then +1 on DVE tensor_scalar)
                    s0 = work.tile([128, 512], F32, tag="S0")
                    nc.gpsimd.tensor_add(s0[:], sp[:], e_all[:, 1024:])
                    s_f = work.tile([128, 512], F32, tag="S")
                    nc.vector.tensor_scalar_add(s_f[:], s0[:], 1.0)
                    # custom NR reciprocal writes aff_0 (bf16) directly
                    aff = work.tile([128, 4 * 512], EW_DT, tag="aff")
                    nc.vector._custom_dve(
                        _RF,
                        out=aff[:, :512],
                        in0=s_f[:],
                        s0=_RC["s0"],
                        s1=_RC["s1"],
                        imm2=_RC["imm2"],
                    )
                    nc.vector.tensor_mul(
                        aff[:, 512:].rearrange("p (h m) -> p h m", h=3),
                        e_all.rearrange("p (h m) -> p h m", h=3),
                        aff[:, None, :512].to_broadcast((128, 3, 512)),
                    )
                    aff_tiles[it] = aff
                # --- readout: LAG iterations behind, accumulate over nt ---
                ro = it - LAG
                if 0 <= ro < NITER:
                    mh_r, nt_r = divmod(ro, NT)
                    if nt_r == 0:
                        mem_tiles[mh_r] = memp.tile(
                            [128, 4 * 512], F32, tag="mem", name=f"mem_ps{mh_r}"
                        )
                    mem_ps = mem_tiles[mh_r]
                    aff_r = aff_tiles.pop(ro)
                    for h in range(HEADS):
                        nc.tensor.matmul(
                            mem_ps[:, h * 512 : (h + 1) * 512],
                            lhsT=mvt_sb[:, nt_r * CV + h * 128 : nt_r * CV + h * 128 + 128],
                            rhs=aff_r[:, h * 512 : (h + 1) * 512],
                            start=(nt_r == 0),
                            stop=(nt_r == NT - 1),
                        )
                    if nt_r == NT - 1:
                        mem_sb = outp.tile([128, 4 * 512], F32)
                        for h in range(HEADS):
                            # per-head copy, alternating ACT/DVE, so each
                            # output DMA starts as soon as its slice is
                            # staged (shorter kernel tail)
                            dst = mem_sb[:, h * 512 : (h + 1) * 512]
                            src = mem_ps[:, h * 512 : (h + 1) * 512]
                            if h % 2 == 0:
                                nc.scalar.activation(dst, src, Copy)
                            else:
                                nc.vector.tensor_copy(dst, src)
                            nc.sync.dma_start(
                                out=mem_d[h * 128 : (h + 1) * 128,
                                          mh_r * 512 : (mh_r + 1) * 512],
                                in_=mem_sb[:, h * 512 : (h + 1) * 512],
                            )
    return nc


def host_decompose(mk, qk, ms, qe, mv):
    """Build the 8 per-core input dicts."""
    mk_f = np.asarray(mk, np.float32).reshape(B, CK, THW)
    mv_f = np.asarray(mv, np.float32).reshape(B, CV, THW)
    ms_f = np.asarray(ms, np.float32).reshape(B, THW)
    qk_h = np.asarray(qk, np.float32).reshape(B, HEADS, C, HW)
    qe_h = np.asarray(qe, np.float32).reshape(B, HEADS, C, HW)

    msn = ms_f / np.float32(np.sqrt(CK))                       # [B, THW]
    mk3 = (mk_f * mk_f * mk_f).reshape(B, HEADS, C, THW)
    mk_h = mk_f.reshape(B, HEADS, C, THW)
    b_sq = np.einsum("bhcm,bhcm->bhm", qe_h, qk_h**3)          # [B, h, HW]

    sim_np = _np_dt(SIM_DT)
    mvt_np = _np_dt(EW_DT)
    # x rows shared across cores; per-core slice taken below
    x_all = np.empty((B, 3, KD, THW), np.float32)
    w_all = np.empty((B, 3, KD, HW), np.float32)
    for dh, h in enumerate((1, 2, 3)):
        x_all[:, dh, 0:C] = mk3[:, h] * msn[:, None, :]
        x_all[:, dh, C : 2 * C] = mk_h[:, h] * msn[:, None, :]
        x_all[:, dh, 2 * C : 3 * C] = mk3[:, 0] * msn[:, None, :]
        x_all[:, dh, 3 * C : 4 * C] = mk_h[:, 0] * msn[:, None, :]
        x_all[:, dh, 4 * C] = msn
        w_all[:, dh, 0:C] = -qe_h[:, h]
        w_all[:, dh, C : 2 * C] = 2.0 * qk_h[:, h] * qe_h[:, h]
        w_all[:, dh, 2 * C : 3 * C] = qe_h[:, 0]
        w_all[:, dh, 3 * C : 4 * C] = -2.0 * qk_h[:, 0] * qe_h[:, 0]
        w_all[:, dh, 4 * C] = b_sq[:, 0] - b_sq[:, h]

    in_maps = []
    for core in range(8):
        b, j = core // 4, core % 4
        sl = slice(j * NCHUNK, (j + 1) * NCHUNK)
        xw = np.empty((KD, 3 * PB), sim_np)
        for dh in range(3):
            xw[:, dh * PB : dh * PB + NCHUNK] = x_all[b, dh, :, sl].astype(sim_np)
            xw[:, dh * PB + NCHUNK : (dh + 1) * PB] = w_all[b, dh].astype(sim_np)
        mvt = np.ascontiguousarray(mv_f[b, :, sl].T).astype(mvt_np)
        in_maps.append({"xw": xw, "mvt": mvt})
    return in_maps


_NC_CACHE = None


def _get_nc():
    global _NC_CACHE
    if _NC_CACHE is None:
        nc = build_bass()
        if not nc.is_finalized():
            nc.finalize()  # Bacc compile: wait legalization etc.
        _NC_CACHE = nc
    return _NC_CACHE


def kernel(mk, qk, ms, qe, mv, qv, _trace=False, _trace_kwargs=None):
    in_maps = host_decompose(mk, qk, ms, qe, mv)
    nc = _get_nc()
    res = run_bass_kernel_spmd(
        nc, in_maps, list(range(8)), trace=_trace, **(_trace_kwargs or {})
    )
    mem = np.zeros((B, CV, HW), np.float32)
    for core in range(8):
        mem[core // 4] += res.results[core]["mem"]
    out = np.concatenate(
        [mem.reshape(B, CV, H, W), np.asarray(qv, np.float32).reshape(B, CV, H, W)],
        axis=1,
    )
    if _trace:
        return out, res
    return out


# revision 18
# speedup vs baseline: 1.1963x; 1.0702x over previous
"""Trainium2 Bass kernel for nn_MemoryReader (retrieval_knn).

Math (per batch b):
  mk_h [h,c,n] (c=16, n=THW=8192), qk_h/qe_h [h,c,m] (m=HW=1024)
  logits[h,n,m] = (ms[n]/8) * ( sum_c mk^3*(-qe) + mk*(2*qk*qe) + (-b_sq) )
  aff = softmax over h
  mem[h,c',m] = sum_n mo[h,c',n] * aff[h,n,m]   (c'=128)
  out = concat(mem, qv)

Sharding: 8 cores = 2 batches x 4 THW-chunks (n-chunk 2048/core). Softmax is
over heads -> core-local. Readout partial-sums over n are reduced on host
during the gather (legit unshard of a contraction-sharded axis).

3-exp diff formulation: softmax is shift-invariant, so subtract head 0's
logits on the PE: d_h = sim_h - sim_0 (h=1..3) via K=65 matmuls
  x rows: [mk^3_h*msn(16); mk_h*msn(16); mk^3_0*msn(16); mk_0*msn(16); msn]
  w rows: [-qe_h; 2qk_h qe_h; +qe_0; -2qk_0 qe_0; b_0-b_h]
then aff_0 = 1/(1 + e^d1 + e^d2 + e^d3) comes straight out of the DVE
reciprocal (no mul for head 0), and only 3 exps run on ACT (the pacing
engine) instead of 4.

PSUM (8 banks): mem accumulate 4 + D12 2 + D3 1 + warm 1. The warm bank is
a dependency-free drip-heater target: one small MM per iteration keeps the
PE's HAM activity monitor at K=8/8 (2.4GHz) even across the exp-wait gaps
(it has no producers/consumers, so it runs exactly when the PE idles).

Pipeline: readout matmuls are emitted LAG=3 iterations behind the softmax
chain that produces their aff operand, so the PE never stalls on the
ACT->DVE->GPS->DVE chain latency.
"""

import sys

sys.path.insert(0, "/opt/trn_rl_repo")

import numpy as np

import concourse.bass as bass
import concourse.tile as tile
from concourse import bacc, mybir
from concourse.bass_utils import run_bass_kernel_spmd

try:
    import ml_dtypes

    _BF16_NP = np.dtype(ml_dtypes.bfloat16)
except ImportError:  # pragma: no cover
    _BF16_NP = None

HEADS, B, CK, CV = 4, 2, 64, 512
T, H, W = 8, 32, 32
THW, HW = T * H * W, H * W          # 8192, 1024
C = CK // HEADS                      # 16
NCHUNK = THW // 4                    # 2048 n per core
NT = NCHUNK // 128                   # 16 n-tiles per core
KD = 4 * C + 1                       # 65 rows per diff matmul
NITER = 2 * NT                       # (mh, nt) flat iteration count
LAG = 4                              # readout lag (iterations)
PB = NCHUNK + HW                     # per-diff block: [X 2048 | W 1024]

F32 = mybir.dt.float32
BF16 = mybir.dt.bfloat16

SIM_DT = BF16            # x/w dtype (bf16: 1 cyc/col at every PE p-state)
EW_DT = BF16             # dtype of e/aff (softmax elementwise) + mvt


def _np_dt(dt):
    return _BF16_NP if dt == BF16 else np.float32


def build_bass():
    # Bacc (not plain Bass): its compile()/finalize() pipeline legalizes
    # multi-wait instructions (TRN2 allows 1 wait/inst) via event semaphores.
    nc = bacc.Bacc(None)
    xw_d = nc.dram_tensor("xw", [KD, 3 * PB], SIM_DT, kind="ExternalInput")
    mvt_d = nc.dram_tensor("mvt", [NCHUNK, CV], EW_DT, kind="ExternalInput")
    mem_d = nc.dram_tensor("mem", [CV, HW], F32, kind="ExternalOutput")

    Exp = mybir.ActivationFunctionType.Exp
    Copy = mybir.ActivationFunctionType.Copy
    Add = mybir.AluOpType.add

    from concourse.dve_ops import (
        RECIP_APPROX_FAST_CONSTS as _RC,
        RECIPROCAL_APPROX_FAST as _RF,
    )

    with tile.TileContext(nc) as tc:
        with (
            tc.tile_pool(name="const", bufs=1) as constp,
            tc.tile_pool(name="simp", bufs=1, space="PSUM") as simp,
            tc.tile_pool(name="memp", bufs=1, space="PSUM") as memp,
            tc.tile_pool(name="work", bufs=6) as work,
            tc.tile_pool(name="outp", bufs=2) as outp,
        ):
            xw_sb = constp.tile([KD, 3 * PB], SIM_DT)
            # W blocks first so the first iteration's streams arrive early,
            # then X in nt-order quarters interleaved across the 3 diffs.
            for dh in range(3):
                o = dh * PB + NCHUNK
                nc.sync.dma_start(out=xw_sb[:, o : o + HW], in_=xw_d[:, o : o + HW])
            for xq in range(4):
                for dh in range(3):
                    o = dh * PB + xq * (NCHUNK // 4)
                    nc.sync.dma_start(
                        out=xw_sb[:, o : o + NCHUNK // 4],
                        in_=xw_d[:, o : o + NCHUNK // 4],
                    )
            mvt_sb = constp.tile([128, NT * CV], EW_DT)
            for nt in range(NT):
                nc.sync.dma_start(
                    out=mvt_sb[:, nt * CV : (nt + 1) * CV],
                    in_=mvt_d[nt * 128 : (nt + 1) * 128, :],
                )

            # Startup heater + dedicated drip bank (no producers/consumers:
            # drip MMs only WAW-chain among themselves on the PE queue).
            hsrc = constp.tile([64, 768], BF16)
            nc.vector.memset(hsrc[:], 0.0)
            warm = simp.tile([128, 512], F32, tag="warm")
            for _ in range(10):
                wmm = nc.tensor.matmul(
                    warm[:],
                    lhsT=hsrc[:, 0:128],
                    rhs=hsrc[:, 128:640],
                    start=True,
                    stop=True,
                    tile_position=(0, 0),
                )
                wmm.ins.bass_priority = -100  # pin to the front of the PE queue

            aff_tiles = {}
            mem_tiles = {}
            for it in range(NITER + LAG):
                if it < NITER:
                    mh, nt = divmod(it, NT)
                    # dependency-free drip MM (see module docstring)
                    nc.tensor.matmul(
                        warm[:, 0:128],
                        lhsT=hsrc[:, 0:128],
                        rhs=hsrc[:, 128:256],
                        start=True,
                        stop=True,
                        tile_position=(0, 0),
                    )
                    # --- diff logits d_h = sim_h - sim_0: 3 MMs, K=65 ---
                    d12 = simp.tile([128, 1024], F32, tag="d12")
                    d3 = simp.tile([128, 512], F32, tag="d3")
                    for dh in range(3):
                        out_sl = d12[:, dh * 512 : dh * 512 + 512] if dh < 2 else d3[:]
                        nc.tensor.matmul(
                            out_sl,
                            lhsT=xw_sb[:, dh * PB + nt * 128 : dh * PB + nt * 128 + 128],
                            rhs=xw_sb[:, dh * PB + NCHUNK + mh * 512 : dh * PB + NCHUNK + mh * 512 + 512],
                            start=True,
                            stop=True,
                        )
                    # --- softmax over heads (aff_0 = 1/(1+sum e^d)) ---
                    e_all = work.tile([128, 1536], EW_DT, tag="e")
                    nc.scalar.activation(e_all[:, :1024], d12[:], Exp)
                    nc.scalar.activation(e_all[:, 1024:], d3[:], Exp)
                    sp = work.tile([128, 512], EW_DT, tag="sp")
                    nc.vector.tensor_add(sp[:], e_all[:, :512], e_all[:, 512:1024])
                    # s = 1 + sp + e3 (GPS add; +1 as a bf16 4x tensor_scalar
                    # on DVE — all-bf16 keeps every DVE op at its top tier)
                    s0 = work.tile([128, 512], EW_DT, tag="S0")
                    nc.gpsimd.tensor_add(s0[:], sp[:], e_all[:, 1024:])
                    s_f = work.tile([128, 512], EW_DT, tag="S")
                    nc.vector.tensor_scalar_add(s_f[:], s0[:], 1.0)
                    # custom NR reciprocal writes aff_0 (bf16) directly
                    aff = work.tile([128, 4 * 512], EW_DT, tag="aff")
                    nc.vector._custom_dve(
                        _RF,
                        out=aff[:, :512],
                        in0=s_f[:],
                        s0=_RC["s0"],
                        s1=_RC["s1"],
                        imm2=_RC["imm2"],
                    )
                    nc.vector.tensor_mul(
                        aff[:, 512:].rearrange("p (h m) -> p h m", h=3),
                        e_all.rearrange("p (h m) -> p h m", h=3),
                        aff[:, None, :512].to_broadcast((128, 3, 512)),
                    )
                    aff_tiles[it] = aff
                # --- readout: LAG iterations behind, accumulate over nt ---
                ro = it - LAG
                if 0 <= ro < NITER:
                    mh_r, nt_r = divmod(ro, NT)
                    if nt_r == 0:
                        mem_tiles[mh_r] = memp.tile(
                            [128, 4 * 512], F32, tag="mem", name=f"mem_ps{mh_r}"
                        )
                    mem_ps = mem_tiles[mh_r]
                    aff_r = aff_tiles.pop(ro)
                    for h in range(HEADS):
                        nc.tensor.matmul(
                            mem_ps[:, h * 512 : (h + 1) * 512],
                            lhsT=mvt_sb[:, nt_r * CV + h * 128 : nt_r * CV + h * 128 + 128],
                            rhs=aff_r[:, h * 512 : (h + 1) * 512],
                            start=(nt_r == 0),
                            stop=(nt_r == NT - 1),
                        )
                    if nt_r == NT - 1:
                        mem_sb = outp.tile([128, 4 * 512], F32)
                        for h in range(HEADS):
                            # per-head copy so each output DMA starts as
                            # soon as its slice is staged; all on ACT, which
                            # has slack in the 3-exp design (DVE is pacing)
                            dst = mem_sb[:, h * 512 : (h + 1) * 512]
                            src = mem_ps[:, h * 512 : (h + 1) * 512]
                            nc.scalar.activation(dst, src, Copy)
                            nc.sync.dma_start(
                                out=mem_d[h * 128 : (h + 1) * 128,
                                          mh_r * 512 : (mh_r + 1) * 512],
                                in_=mem_sb[:, h * 512 : (h + 1) * 512],
                            )
    return nc


def host_decompose(mk, qk, ms, qe, mv):
    """Build the 8 per-core input dicts."""
    mk_f = np.asarray(mk, np.float32).reshape(B, CK, THW)
    mv_f = np.asarray(mv, np.float32).reshape(B, CV, THW)
    ms_f = np.asarray(ms, np.float32).reshape(B, THW)
    qk_h = np.asarray(qk, np.float32).reshape(B, HEADS, C, HW)
    qe_h = np.asarray(qe, np.float32).reshape(B, HEADS, C, HW)

    msn = ms_f / np.float32(np.sqrt(CK))                       # [B, THW]
    mk3 = (mk_f * mk_f * mk_f).reshape(B, HEADS, C, THW)
    mk_h = mk_f.reshape(B, HEADS, C, THW)
    b_sq = np.einsum("bhcm,bhcm->bhm", qe_h, qk_h**3)          # [B, h, HW]

    sim_np = _np_dt(SIM_DT)
    mvt_np = _np_dt(EW_DT)
    # x rows shared across cores; per-core slice taken below
    x_all = np.empty((B, 3, KD, THW), np.float32)
    w_all = np.empty((B, 3, KD, HW), np.float32)
    for dh, h in enumerate((1, 2, 3)):
        x_all[:, dh, 0:C] = mk3[:, h] * msn[:, None, :]
        x_all[:, dh, C : 2 * C] = mk_h[:, h] * msn[:, None, :]
        x_all[:, dh, 2 * C : 3 * C] = mk3[:, 0] * msn[:, None, :]
        x_all[:, dh, 3 * C : 4 * C] = mk_h[:, 0] * msn[:, None, :]
        x_all[:, dh, 4 * C] = msn
        w_all[:, dh, 0:C] = -qe_h[:, h]
        w_all[:, dh, C : 2 * C] = 2.0 * qk_h[:, h] * qe_h[:, h]
        w_all[:, dh, 2 * C : 3 * C] = qe_h[:, 0]
        w_all[:, dh, 3 * C : 4 * C] = -2.0 * qk_h[:, 0] * qe_h[:, 0]
        w_all[:, dh, 4 * C] = b_sq[:, 0] - b_sq[:, h]

    in_maps = []
    for core in range(8):
        b, j = core // 4, core % 4
        sl = slice(j * NCHUNK, (j + 1) * NCHUNK)
        xw = np.empty((KD, 3 * PB), sim_np)
        for dh in range(3):
            xw[:, dh * PB : dh * PB + NCHUNK] = x_all[b, dh, :, sl].astype(sim_np)
            xw[:, dh * PB + NCHUNK : (dh + 1) * PB] = w_all[b, dh].astype(sim_np)
        mvt = np.ascontiguousarray(mv_f[b, :, sl].T).astype(mvt_np)
        in_maps.append({"xw": xw, "mvt": mvt})
    return in_maps


_NC_CACHE = None


def _get_nc():
    global _NC_CACHE
    if _NC_CACHE is None:
        nc = build_bass()
        if not nc.is_finalized():
            nc.finalize()  # Bacc compile: wait legalization etc.
        _NC_CACHE = nc
    return _NC_CACHE


def kernel(mk, qk, ms, qe, mv, qv, _trace=False, _trace_kwargs=None):
    in_maps = host_decompose(mk, qk, ms, qe, mv)
    nc = _get_nc()
    res = run_bass_kernel_spmd(
        nc, in_maps, list(range(8)), trace=_trace, **(_trace_kwargs or {})
    )
    mem = np.zeros((B, CV, HW), np.float32)
    for core in range(8):
        mem[core // 4] += res.results[core]["mem"]
    out = np.concatenate(
        [mem.reshape(B, CV, H, W), np.asarray(qv, np.float32).reshape(B, CV, H, W)],
        axis=1,
    )
    if _trace:
        return out, res
    return out
